# revision 1
# baseline (speedup 1.0000x reference)
"""Trainium2 Bass kernel for GCBlockP1 (GNN message passing block).

Computation (reference):
    h = tanh(tanh(p1 @ pp_w1 + pp_b1) @ pp_w2 + pp_b2)          [N, D]
    inter = concat(h[idx_i], h[idx_j]) @ pi_w + pi_b            [E, D*B]
    inter = einsum('pcb,pb->pc', inter.reshape(E, D, B), basis) [E, D]
    i1 = tanh(inter @ ii_w + ii_b)                              [E, D]
    out = segment_sum(i1, idx_j, N)                             [N, D]

Strategy (8 NeuronCores, SPMD, zero collectives):
  - Host sorts edges by idx_j, splits into 8 contiguous destination-node
    ranges (~E/8 edges each). Each core only produces node rows in its own
    range, so results concatenate with a trivial host-side overlap-add.
  - Each core recomputes the (small) node MLP for all nodes in bf16 and
    stores h row-major in DRAM.
  - Edges are processed in groups of 16 chunks x 128 edges. Groups are cut
    so all destination nodes of a group fit a 512-node window; short groups
    are padded (pad edges carry jrel=-1 so they never scatter).
  - Per 128-edge chunk: indirect-DMA row gathers of h[idx_i], h[idx_j];
    DMA-transpose; two K=128 bf16 matmuls -> PSUM [e, D*B]; ACT evicts to
    bf16; DVE multiplies by broadcast basis and group-of-8-reduces (the
    einsum); DMA-transpose; ii matmul; ACT tanh; scatter via an is_equal
    segment-indicator matmul accumulating [d, 512] in PSUM across the
    group's 16 chunks; PSUM -> DRAM staging per group.
  - Host adds staging slabs into the output at each group's base node.
"""

import numpy as np
import ml_dtypes

import concourse.bass as bass
import concourse.bacc as bacc
import concourse.mybir as mybir
import concourse.tile as tile
from concourse.bass_utils import run_bass_kernel_spmd

BF16 = ml_dtypes.bfloat16

NCORES = 8
D = 128
NB = 8
CHUNK = 128          # edges per chunk (one SBUF partition set)
CPG = 16             # chunks per group
GROUP = CHUNK * CPG  # 2048 edge slots per group
WIN = 512            # destination-node window per group
PPT = 512            # nodes per pp-phase tile


# ---------------------------------------------------------------------------
# Host-side planning
# ---------------------------------------------------------------------------

def _plan(idx_i, idx_j, basis, n_nodes, ncores):
    """Sort edges by destination, split across cores at node boundaries,
    cut into (<=GROUP edges, <=WIN node-span) groups, pack device arrays."""
    E = idx_i.shape[0]
    order = np.argsort(idx_j, kind="stable")
    ji = idx_j[order]

    starts = [0]
    for c in range(1, ncores):
        pos = c * E // ncores
        pos = int(np.searchsorted(ji, ji[pos], side="left"))
        starts.append(pos)
    starts.append(E)

    per_core_groups = []
    for c in range(ncores):
        lo, hi = starts[c], starts[c + 1]
        jc = ji[lo:hi]
        oc = order[lo:hi]
        groups = []
        p = 0
        while p < len(jc):
            base = int(jc[p])
            pend = min(p + GROUP, len(jc))
            pend = min(pend, int(np.searchsorted(jc, base + WIN, side="left")))
            groups.append((base, oc[p:pend]))
            p = pend
        per_core_groups.append(groups)

    G = max(len(g) for g in per_core_groups)

    cores = []
    for c in range(ncores):
        groups = per_core_groups[c]
        gi = np.zeros((G, GROUP), np.int32)
        gj = np.zeros((G, GROUP), np.int32)
        jr = np.full((G, GROUP), -1, np.float32)
        bs = np.zeros((G, GROUP, NB), np.float32)
        bases = np.zeros(G, np.int64)
        for g, (base, sel) in enumerate(groups):
            n = len(sel)
            gi[g, :n] = idx_i[sel]
            gj[g, :n] = idx_j[sel]
            jr[g, :n] = (idx_j[sel] - base).astype(np.float32)
            bs[g, :n] = basis[sel]
            bases[g] = base
        # device layout: slot (q, p) = edge q*CHUNK+p  ->  [G, p(128), q(16)]
        # merged gather offsets: [G, p(128), q(16), 2] with (idx_i, idx_j)
        gij = np.stack([gi.reshape(G, CPG, CHUNK).transpose(0, 2, 1),
                        gj.reshape(G, CPG, CHUNK).transpose(0, 2, 1)], axis=-1)
        cores.append(dict(
            gij=np.ascontiguousarray(gij),
            jr=np.ascontiguousarray(jr.reshape(G, CPG, CHUNK).transpose(0, 2, 1)),
            bs=np.ascontiguousarray(
                bs.reshape(G, CPG, CHUNK, NB).transpose(0, 2, 1, 3)).astype(BF16),
            bases=bases,
            ngroups=len(groups),
        ))
    return cores, G


# ---------------------------------------------------------------------------
# Device program
# ---------------------------------------------------------------------------

def _bcast_mid(ap, count):
    """[P, k] AP -> [P, count, k] AP with a stride-0 middle dim."""
    return bass.AP(ap.tensor, ap.offset, [ap.ap[0], [0, count], ap.ap[1]])


def _build(npad, G, nz_pib, nz_iib, repeat=1):
    nc = bacc.Bacc("TRN2", num_swdge_queues=2)
    f32, bf16 = mybir.dt.float32, mybir.dt.bfloat16
    i32, i16 = mybir.dt.int32, mybir.dt.int16

    p1b = nc.dram_tensor("p1b", [npad, D], bf16, kind="ExternalInput")
    w1 = nc.dram_tensor("w1", [D, D], bf16, kind="ExternalInput")
    w2 = nc.dram_tensor("w2", [D, D], bf16, kind="ExternalInput")
    b1 = nc.dram_tensor("b1", [D, 1], f32, kind="ExternalInput")
    b2 = nc.dram_tensor("b2", [D, 1], f32, kind="ExternalInput")
    piwi = nc.dram_tensor("piwi", [D, D * NB], bf16, kind="ExternalInput")
    piwj = nc.dram_tensor("piwj", [D, D * NB], bf16, kind="ExternalInput")
    iiw = nc.dram_tensor("iiw", [D, D], bf16, kind="ExternalInput")
    gij = nc.dram_tensor("gij", [G, CHUNK, CPG, 2], i32, kind="ExternalInput")
    jr = nc.dram_tensor("jr", [G, CHUNK, CPG], f32, kind="ExternalInput")
    bas = nc.dram_tensor("bas", [G, CHUNK, CPG, NB], bf16, kind="ExternalInput")
    if nz_pib:
        pibr = nc.dram_tensor("pibr", [CHUNK, D * NB], bf16, kind="ExternalInput")
    if nz_iib:
        iibr = nc.dram_tensor("iibr", [CHUNK, D], bf16, kind="ExternalInput")

    staging = nc.dram_tensor("staging", [G, D, WIN], f32, kind="ExternalOutput")
    h_dram = nc.dram_tensor("h_dram", [npad, D], bf16)

    npp = npad // PPT

    with tile.TileContext(nc) as tc:
        with tc.tile_pool(name="const", bufs=1) as cpool:
            w1_t = cpool.tile([D, D], bf16)
            w2_t = cpool.tile([D, D], bf16)
            b1_t = cpool.tile([D, 1], f32)
            b2_t = cpool.tile([D, 1], f32)
            piwi_t = cpool.tile([D, D * NB], bf16)
            piwj_t = cpool.tile([D, D * NB], bf16)
            iiw_t = cpool.tile([D, D], bf16)
            iota_t = cpool.tile([CHUNK, WIN], i16)
            nc.sync.dma_start(out=w1_t[:], in_=w1[:])
            nc.sync.dma_start(out=w2_t[:], in_=w2[:])
            nc.sync.dma_start(out=b1_t[:], in_=b1[:])
            nc.sync.dma_start(out=b2_t[:], in_=b2[:])
            nc.sync.dma_start(out=piwi_t[:], in_=piwi[:])
            nc.sync.dma_start(out=piwj_t[:], in_=piwj[:])
            nc.sync.dma_start(out=iiw_t[:], in_=iiw[:])
            nc.gpsimd.iota(iota_t[:], [[1, WIN]], channel_multiplier=0)
            if nz_pib:
                pibr_t = cpool.tile([CHUNK, D * NB], bf16)
                nc.sync.dma_start(out=pibr_t[:], in_=pibr[:])
            if nz_iib:
                iibr_t = cpool.tile([CHUNK, D], bf16)
                nc.sync.dma_start(out=iibr_t[:], in_=iibr[:])

            for _rep in range(repeat):
                _build_phases(nc, tc, locals())
    nc.compile()
    return nc


def _build_phases(nc, tc, env):
    (f32, bf16, i32, i16) = (mybir.dt.float32, mybir.dt.bfloat16,
                             mybir.dt.int32, mybir.dt.int16)
    w1_t = env["w1_t"]; w2_t = env["w2_t"]; b1_t = env["b1_t"]
    b2_t = env["b2_t"]; piwi_t = env["piwi_t"]; piwj_t = env["piwj_t"]
    iiw_t = env["iiw_t"]; iota_t = env["iota_t"]
    p1b = env["p1b"]; h_dram = env["h_dram"]; npp = env["npp"]
    gij = env["gij"]; jr = env["jr"]; bas = env["bas"]
    staging = env["staging"]; G = env["G"]
    nz_pib = env["nz_pib"]; nz_iib = env["nz_iib"]
    pibr_t = env.get("pibr_t"); iibr_t = env.get("iibr_t")
    if True:
        if True:
            # ---- phase 1: node MLP, h = tanh(tanh(p1@w1+b1)@w2+b2) ----
            # h-row writes for tile t are deferred to iteration t+1 so the
            # in-order SP stream never stalls on tile t's tanh.
            with tc.tile_pool(name="pp", bufs=4) as pp, \
                 tc.tile_pool(name="ppp", bufs=2, space="PSUM") as ppp:
                def pp_writes(h2, t):
                    for qq in range(PPT // D):
                        hr = pp.tile([D, D], bf16, name="hr", tag="hr")
                        nc.sync.dma_start_transpose(
                            hr[:], h2[:, qq * D:(qq + 1) * D])
                        nc.sync.dma_start(
                            out=h_dram[t * PPT + qq * D:t * PPT + (qq + 1) * D, :],
                            in_=hr[:])

                prev = None
                for t in range(npp):
                    p1T = pp.tile([D, PPT], bf16)
                    nc.sync.dma_start_transpose(
                        p1T[:], p1b[t * PPT:(t + 1) * PPT, :])
                    ps1 = ppp.tile([D, PPT], f32)
                    nc.tensor.matmul(out=ps1[:], lhsT=w1_t[:], rhs=p1T[:],
                                     start=True, stop=True)
                    h1 = pp.tile([D, PPT], bf16)
                    nc.scalar.activation(h1[:], ps1[:],
                                         mybir.ActivationFunctionType.Tanh,
                                         bias=b1_t[:, :1])
                    ps2 = ppp.tile([D, PPT], f32)
                    nc.tensor.matmul(out=ps2[:], lhsT=w2_t[:], rhs=h1[:],
                                     start=True, stop=True)
                    h2 = pp.tile([D, PPT], bf16)
                    nc.scalar.activation(h2[:], ps2[:],
                                         mybir.ActivationFunctionType.Tanh,
                                         bias=b2_t[:, :1])
                    if prev is not None:
                        pp_writes(*prev)
                    prev = (h2, t)
                pp_writes(*prev)

            # ---- phase 2: edges ----
            # Flat chunk pipeline, software-pipelined so every consumer runs
            # well after its producer's latency: the ii matmul for chunk k is
            # emitted at k+DELAY_II, the scatter matmul at k+DELAY_SC (one
            # full group), so the in-order PE stream never stalls.
            DELAY_EIN = 3
            DELAY_S = 3
            DELAY_IRT = 5
            DELAY_II = 10
            DELAY_SC = CPG
            K = G * CPG
            with tc.tile_pool(name="eg", bufs=3) as eg, \
                 tc.tile_pool(name="ew", bufs=12) as ew, \
                 tc.tile_pool(name="late", bufs=DELAY_SC + 14) as late, \
                 tc.tile_pool(name="psI", bufs=2, space="PSUM") as psIp, \
                 tc.tile_pool(name="psJ", bufs=2, space="PSUM") as psJp, \
                 tc.tile_pool(name="psA", bufs=2, space="PSUM") as psAp:
                groups = {}   # g -> (git, jrt, bat)
                gaths = {}    # k -> (tile, col offset)
                sbIs = {}     # k -> evicted inter (awaiting einsum)
                ireds = {}    # k -> ired tile (awaiting irT transpose)
                irTs = {}     # k -> transposed ired (awaiting ii matmul)
                i1s = {}      # k -> i1 tile (awaiting scatter)
                Ss = {}       # k -> S tile (awaiting scatter)
                psAs = {}     # g -> psum accumulator

                def load_group(g):
                    git = eg.tile([CHUNK, CPG * 2], i32)
                    jrt = eg.tile([CHUNK, CPG], f32)
                    bat = eg.tile([CHUNK, CPG * NB], bf16)
                    nc.sync.dma_start(
                        out=git[:], in_=gij[g].rearrange("p q t -> p (q t)"))
                    nc.sync.dma_start(out=jrt[:], in_=jr[g])
                    nc.sync.dma_start(
                        out=bat[:], in_=bas[g].rearrange("p q b -> p (q b)"))
                    groups[g] = (git, jrt, bat)

                def issue_gather_pair(k):
                    for kk in (k, k + 1):
                        g, q = divmod(kk, CPG)
                        git = groups[g][0]
                        gath = ew.tile([CHUNK, 2 * D], bf16, name="gath",
                                       tag="gath", bufs=20)
                        g1 = nc.gpsimd.indirect_dma_start(
                            out=gath[:, 0:D], out_offset=None, in_=h_dram[:],
                            in_offset=bass.IndirectOffsetOnAxis(
                                ap=git[:, 2 * q:2 * q + 1], axis=0))
                        g2 = nc.gpsimd.indirect_dma_start(
                            out=gath[:, D:2 * D], out_offset=None, in_=h_dram[:],
                            in_offset=bass.IndirectOffsetOnAxis(
                                ap=git[:, 2 * q + 1:2 * q + 2], axis=0))
                        g2.ins.queue = "qPoolDynamic1"
                        gaths[kk] = (gath, 0)

                hTs = {}

                def issue_transposes(k):
                    gath, off = gaths.pop(k)
                    hTi = ew.tile([D, CHUNK], bf16, name="hTi", tag="hTi")
                    hTj = ew.tile([D, CHUNK], bf16, name="hTj", tag="hTj")
                    nc.sync.dma_start_transpose(hTi[:], gath[:, off:off + D])
                    nc.sync.dma_start_transpose(
                        hTj[:], gath[:, off + D:off + 2 * D])
                    hTs[k] = (hTi, hTj)

                def compute_chunk(k):
                    g, q = divmod(k, CPG)
                    git, jrt, bat = groups[g]
                    hTi, hTj = hTs.pop(k)

                    psI = psIp.tile([CHUNK, D * NB], f32)
                    half = D * NB // 2
                    nc.tensor.matmul(out=psI[:, 0:half], lhsT=hTi[:],
                                     rhs=piwi_t[:, 0:half],
                                     start=True, stop=False)
                    nc.tensor.matmul(out=psI[:, half:], lhsT=hTi[:],
                                     rhs=piwi_t[:, half:],
                                     start=True, stop=False)
                    nc.tensor.matmul(out=psI[:, 0:half], lhsT=hTj[:],
                                     rhs=piwj_t[:, 0:half],
                                     start=False, stop=True)
                    nc.tensor.matmul(out=psI[:, half:], lhsT=hTj[:],
                                     rhs=piwj_t[:, half:],
                                     start=False, stop=True)

                    sbI = late.tile([CHUNK, D * NB], bf16, tag="sbI", bufs=8)
                    nc.scalar.activation(sbI[:], psI[:],
                                         mybir.ActivationFunctionType.Copy)
                    if nz_pib:
                        nc.vector.tensor_tensor(
                            out=sbI[:], in0=sbI[:], in1=pibr_t[:],
                            op=mybir.AluOpType.add)
                    sbIs[k] = sbI

                def einsum_chunk(k):
                    g, q = divmod(k, CPG)
                    git, jrt, bat = groups[g]
                    sbI = sbIs.pop(k)
                    prod = ew.tile([CHUNK, D * NB], bf16)
                    nc.vector.tensor_tensor(
                        out=prod[:], in0=sbI[:],
                        in1=_bcast_mid(bat[:, q * NB:(q + 1) * NB], D),
                        op=mybir.AluOpType.mult)
                    # reduce groups of NB=8 via a 3-level pairwise tree (TT
                    # adds run 2x_1P; tensor_reduce would run 1x).
                    r1 = ew.tile([CHUNK, D * 4], bf16)
                    p3 = prod[:].rearrange("p (c b) -> p c b", b=NB)
                    nc.vector.tensor_tensor(
                        out=r1[:].rearrange("p (c b) -> p c b", b=4),
                        in0=p3[:, :, 0:4], in1=p3[:, :, 4:8],
                        op=mybir.AluOpType.add)
                    r2 = ew.tile([CHUNK, D * 2], bf16)
                    r1v = r1[:].rearrange("p (c b) -> p c b", b=4)
                    nc.vector.tensor_tensor(
                        out=r2[:].rearrange("p (c b) -> p c b", b=2),
                        in0=r1v[:, :, 0:2], in1=r1v[:, :, 2:4],
                        op=mybir.AluOpType.add)
                    ired = late.tile([CHUNK, D], bf16, tag="ired", bufs=8)
                    r2v = r2[:].rearrange("p (c b) -> p c b", b=2)
                    nc.vector.tensor_tensor(
                        out=ired[:], in0=r2v[:, :, 0], in1=r2v[:, :, 1],
                        op=mybir.AluOpType.add)
                    ireds[k] = ired

                def sbuild_chunk(k):
                    g, q = divmod(k, CPG)
                    jrt = groups[g][1]
                    S = late.tile([CHUNK, WIN], bf16, tag="S")
                    nc.vector.tensor_scalar(
                        out=S[:], in0=iota_t[:], scalar1=jrt[:, q:q + 1],
                        scalar2=None, op0=mybir.AluOpType.is_equal)
                    Ss[k] = S

                def irt_chunk(k):
                    ired = ireds.pop(k)
                    irT = late.tile([D, CHUNK], bf16, tag="irT", bufs=12)
                    nc.sync.dma_start_transpose(irT[:], ired[:])
                    irTs[k] = irT

                def ii_chunk(k):
                    irT = irTs.pop(k)
                    psJ = psJp.tile([CHUNK, D], f32)
                    nc.tensor.matmul(out=psJ[:], lhsT=irT[:], rhs=iiw_t[:],
                                     start=True, stop=True)
                    i1 = late.tile([CHUNK, D], bf16, tag="i1")
                    if nz_iib:
                        tmp = late.tile([CHUNK, D], bf16, tag="i1tmp")
                        nc.vector.tensor_tensor(
                            out=tmp[:], in0=psJ[:], in1=iibr_t[:],
                            op=mybir.AluOpType.add)
                        nc.scalar.activation(
                            i1[:], tmp[:], mybir.ActivationFunctionType.Tanh)
                    else:
                        nc.scalar.activation(
                            i1[:], psJ[:], mybir.ActivationFunctionType.Tanh)
                    i1s[k] = i1

                def scatter_chunk(k):
                    g, q = divmod(k, CPG)
                    if q == 0:
                        psAs[g] = psAp.tile([D, WIN], f32, name="psA", tag="psA")
                    nc.tensor.matmul(out=psAs[g][:], lhsT=i1s.pop(k)[:],
                                     rhs=Ss.pop(k)[:],
                                     start=(q == 0), stop=(q == CPG - 1))
                    if q == CPG - 1:
                        psA = psAs.pop(g)
                        acc_sb = eg.tile([D, WIN], f32)
                        nc.scalar.activation(acc_sb[:], psA[:],
                                             mybir.ActivationFunctionType.Copy)
                        nc.sync.dma_start(out=staging[g], in_=acc_sb[:])

                GA = 10
                TA = 2
                load_group(0)
                for j in range(0, min(GA, K), 2):
                    issue_gather_pair(j)
                for j in range(0, min(TA, K)):
                    issue_transposes(j)
                for k in range(K + DELAY_SC):
                    ka = k + GA
                    if ka < K and ka % CPG == 0:
                        load_group(ka // CPG)
                    if ka < K and ka % 2 == 0:
                        issue_gather_pair(ka)
                    if k + TA < K:
                        issue_transposes(k + TA)
                    if k < K:
                        compute_chunk(k)
                    if 0 <= k - DELAY_EIN < K:
                        einsum_chunk(k - DELAY_EIN)
                    if 0 <= k - DELAY_S < K:
                        sbuild_chunk(k - DELAY_S)
                    if 0 <= k - DELAY_IRT < K:
                        irt_chunk(k - DELAY_IRT)
                    if 0 <= k - DELAY_II < K:
                        ii_chunk(k - DELAY_II)
                    if 0 <= k - DELAY_SC < K:
                        scatter_chunk(k - DELAY_SC)


# ---------------------------------------------------------------------------
# Entry point
# ---------------------------------------------------------------------------

def _prep_inputs(p1, idx_i, idx_j, basis, pp_w1, pp_b1, pp_w2, pp_b2,
                 pi_w, pi_b, ii_w, ii_b, ncores):
    n_nodes = p1.shape[0]
    npad = ((n_nodes + PPT - 1) // PPT) * PPT
    p1b = np.zeros((npad, D), BF16)
    p1b[:n_nodes] = p1.astype(BF16)

    cores, G = _plan(np.asarray(idx_i), np.asarray(idx_j), np.asarray(basis),
                     n_nodes, ncores)

    nz_pib = bool(np.any(pi_b != 0))
    nz_iib = bool(np.any(ii_b != 0))

    common = dict(
        p1b=p1b,
        w1=pp_w1.astype(BF16), w2=pp_w2.astype(BF16),
        b1=pp_b1.astype(np.float32).reshape(D, 1),
        b2=pp_b2.astype(np.float32).reshape(D, 1),
        piwi=pi_w[:D].astype(BF16), piwj=pi_w[D:].astype(BF16),
        iiw=ii_w.astype(BF16),
    )
    if nz_pib:
        common["pibr"] = np.tile(pi_b.astype(BF16)[None, :], (CHUNK, 1))
    if nz_iib:
        common["iibr"] = np.tile(ii_b.astype(BF16)[None, :], (CHUNK, 1))

    in_maps = []
    for c in range(ncores):
        m = dict(common)
        m["gij"] = cores[c]["gij"]
        m["jr"] = cores[c]["jr"]
        m["bas"] = cores[c]["bs"]
        in_maps.append(m)
    return in_maps, cores, G, npad, n_nodes, nz_pib, nz_iib


def _assemble(results, cores, n_nodes):
    out = np.zeros((n_nodes, D), np.float32)
    for c, core in enumerate(cores):
        st = results[c]["staging"]
        for g in range(core["ngroups"]):
            base = int(core["bases"][g])
            w = min(WIN, n_nodes - base)
            out[base:base + w] += st[g, :, :w].T
    return out


LAST_RESULTS = None


def kernel(p1, idx_i, idx_j, basis, pp_w1, pp_b1, pp_w2, pp_b2,
           pi_w, pi_b, ii_w, ii_b):
    global LAST_RESULTS
    in_maps, cores, G, npad, n_nodes, nz_pib, nz_iib = _prep_inputs(
        p1, idx_i, idx_j, basis, pp_w1, pp_b1, pp_w2, pp_b2,
        pi_w, pi_b, ii_w, ii_b, NCORES)
    nc = _build(npad, G, nz_pib, nz_iib)
    res = run_bass_kernel_spmd(nc, in_maps, core_ids=list(range(NCORES)))
    LAST_RESULTS = res
    return _assemble(res.results, cores, n_nodes)

